# revision 19
# baseline (speedup 1.0000x reference)
"""LSTM (T=512, B=64, I=H=512) on 8 TRN2 NeuronCores, batch-data-parallel.

Per core (batch slice of 8):
  phase-1 (interleaved): x_proj = x @ W_ih.T + b  -> bf16 ring, 16 steps/chunk
  recurrence, per step:
    PSUM gates G [128, 512] fp32, "quartered" layout: quarter j (h-dims
    128j..128j+128, gate types i,f,o,g along free in 128-blocks) lives at
    partitions 32j+b.
      - 4 slot-select identity matmuls inject x_proj[t] (start=True, M=32
        zero-padded -> initializes all 128 partitions)
      - 4 k-rounds x 4 col-tiled concurrent matmuls accumulate h@W_hh.T
    ACT: sigmoid(i,f,2g~) / sigmoid(o) from PSUM -> bf16 SBUF
    DVE: Q = (sig(2g~)-0.5)*sig(i); GP: M = sig(f)*c;
    DVE: c' = 2Q + M (fp32); ACT: tanh(c')
    per-quarter: DVE ymul h'_q = sig(o)_q*tanh_q -> y ring; PE transpose;
    DVE copy -> hT quarter; next step's k-round r starts right after
    copy-q_r (quarter-pipelined tail).
  output ring DMA'd to HBM bf16 every 64 steps; host converts to fp32.
"""
import numpy as np
import ml_dtypes
from contextlib import ExitStack

import concourse.bass as bass
import concourse.tile as tile
from concourse import bacc, mybir
from concourse._compat import with_exitstack

BF16 = mybir.dt.bfloat16
F32 = mybir.dt.float32
AF = mybir.ActivationFunctionType
ALU = mybir.AluOpType
bf = ml_dtypes.bfloat16

T, B, I, H = 512, 64, 512, 512
NCORES = 8
BL = B // NCORES          # 8 batch rows per core
CH = 16                   # phase-1 chunk (timesteps)
RC = 64                   # output ring chunk (timesteps)
LOOKAHEAD = 2             # xp chunks produced ahead


def build_program(T_=T):
    nch = T_ // CH
    nrc = T_ // RC
    nc = bacc.Bacc("TRN2", target_bir_lowering=False, debug=False)

    xT_d = nc.dram_tensor("xT", [nch, 128, 512], BF16, kind="ExternalInput").ap()
    wih_d = nc.dram_tensor("wih", [128, 4 * 2048], BF16, kind="ExternalInput").ap()
    whh_d = nc.dram_tensor("whh", [128, 4 * 2048], BF16, kind="ExternalInput").ap()
    bias_d = nc.dram_tensor("bias", [1, 2048], BF16, kind="ExternalInput").ap()
    E_d = nc.dram_tensor("E", [128, 512], BF16, kind="ExternalInput").ap()
    I4_d = nc.dram_tensor("I4", [128, 8], BF16, kind="ExternalInput").ap()
    ones_d = nc.dram_tensor("ones", [1, 128], BF16, kind="ExternalInput").ap()
    y_d = nc.dram_tensor("y", [nrc, 4, 8, RC * 128], BF16, kind="ExternalOutput").ap()

    with tile.TileContext(nc) as tc:
        _kernel(tc, y_d, xT_d, wih_d, whh_d, bias_d, E_d, I4_d, ones_d, T_)
    nc.compile()
    return nc


@with_exitstack
def _kernel(ctx: ExitStack, tc: tile.TileContext, y_d, xT_d, wih_d, whh_d,
            bias_d, E_d, I4_d, ones_d, T_):
    nc = tc.nc
    nch = T_ // CH

    const_pool = ctx.enter_context(tc.tile_pool(name="const", bufs=1))
    w_pool = ctx.enter_context(tc.tile_pool(name="weights", bufs=1))
    xT_pool = ctx.enter_context(tc.tile_pool(name="xT", bufs=3))
    xp_pool = ctx.enter_context(tc.tile_pool(name="xp", bufs=LOOKAHEAD + 1))
    y_pool = ctx.enter_context(tc.tile_pool(name="yring", bufs=2))
    ew_pool = ctx.enter_context(tc.tile_pool(name="ew", bufs=2))
    state_pool = ctx.enter_context(tc.tile_pool(name="state", bufs=2))

    g_psum = ctx.enter_context(tc.tile_pool(name="gpsum", bufs=2, space="PSUM"))
    xp_psum = ctx.enter_context(tc.tile_pool(name="xppsum", bufs=2, space="PSUM"))
    tp_psum = ctx.enter_context(tc.tile_pool(name="tppsum", bufs=4, space="PSUM"))

    # ---- load constants/weights ----
    wih_t = w_pool.tile([128, 4 * 2048], BF16)
    nc.sync.dma_start(wih_t[:], wih_d[:])
    whh_t = w_pool.tile([128, 4 * 2048], BF16)
    nc.sync.dma_start(whh_t[:], whh_d[:])
    bias_t = const_pool.tile([1, 2048], BF16)
    nc.sync.dma_start(bias_t[:], bias_d[:])
    E_t = const_pool.tile([128, 512], BF16)
    nc.sync.dma_start(E_t[:], E_d[:])
    I4_t = const_pool.tile([128, 8], BF16)
    nc.sync.dma_start(I4_t[:], I4_d[:])
    ones_t = const_pool.tile([1, 128], BF16)
    nc.sync.dma_start(ones_t[:], ones_d[:])

    # ---- initial state ----
    h_cur = state_pool.tile([128, 32], BF16, tag="hT")
    nc.gpsimd.memset(h_cur[:], 0.0)
    c_cur = state_pool.tile([128, 128], F32, tag="c")
    nc.gpsimd.memset(c_cur[:], 0.0)

    xp_tiles = {}
    xt_tiles = {}
    ph1_mms = []              # pending phase-1 matmul thunks, 1 drained/step

    def xp_dma(ch):
        xt = xT_pool.tile([128, 512], BF16, tag="xchunk")
        nc.sync.dma_start(xt[:], xT_d[ch])
        xt_tiles[ch] = xt

    def queue_xp_chunk(ch):
        """queue phase-1 matmuls for chunk ch: 4 quarters x 5 mms + copy."""
        xp_new = xp_pool.tile([128, 2048], BF16, tag="xp")
        xp_tiles[ch] = xp_new
        for qf in range(4):
            ps_box = []
            g0 = qf * 512

            def mk_mm(k, qf=qf, g0=g0, ch=ch, ps_box=ps_box):
                def run():
                    if k == 0:
                        ps_new = xp_psum.tile([128, 512], F32, tag="xpp")
                        ps_box.append(ps_new)
                    ps = ps_box[0]
                    if k < 4:
                        nc.tensor.matmul(
                            ps[:, :],
                            xt_tiles[ch][:, k * 128:(k + 1) * 128],
                            wih_t[:, 2048 * k + g0: 2048 * k + g0 + 512],
                            start=(k == 0), stop=False,
                        )
                    else:
                        nc.tensor.matmul(
                            ps[:, :], ones_t[:, :], bias_t[:, g0:g0 + 512],
                            start=False, stop=True,
                        )
                        nc.vector.tensor_copy(
                            xp_tiles[ch][:, g0:g0 + 512], ps[:])
                return run

            for k in range(5):
                ph1_mms.append(mk_mm(k))

    def drain_ph1(n):
        for _ in range(min(n, len(ph1_mms))):
            ph1_mms.pop(0)()

    def do_idents(t_next):
        """allocate G psum tile for step t_next + slot-select x_proj inject.
        One [128, 512] tile: cols 0:384 = i,f,g~; cols 384:512 = o. The
        region-based dep tracking lets sigmoid(ifg) ignore o-col writes."""
        s = t_next % CH
        xp = xp_tiles[t_next // CH]
        G = g_psum.tile([128, 512], F32, tag="G")
        for j in range(4):
            nc.tensor.matmul(
                G[32 * j:32 * j + 32, 0:384],
                E_t[:, 32 * s:32 * s + 32],
                xp[:, 512 * j:512 * j + 384],
                start=True, stop=True,
                tile_position=(0, 32 * j),
                skip_group_check=(j > 0),
            )
        # start=False: the ifg idents' start=True already pending-zeroes the
        # full 2KB bank region per partition (ZERO_REGION_SIZE), o included.
        for j in range(4):
            nc.tensor.matmul(
                G[32 * j:32 * j + 32, 384:512],
                E_t[:, 32 * s:32 * s + 32],
                xp[:, 512 * j + 384:512 * (j + 1)],
                start=False, stop=True,
                tile_position=(0, 32 * j),
                skip_group_check=True,
            )
        return G

    # prologue: first two x_proj chunks + idents for step 0
    for ch in range(min(LOOKAHEAD, nch)):
        xp_dma(ch)
        queue_xp_chunk(ch)
        drain_ph1(20)
    G_cur = do_idents(0)

    y_tile = None
    for t in range(T_):
        s = t % CH
        ch = t // CH
        tl = t % RC
        rc = t // RC

        if s == 0 and ch + LOOKAHEAD < nch:
            xp_dma(ch + LOOKAHEAD)
            queue_xp_chunk(ch + LOOKAHEAD)
        if tl == 0:
            y_tile = y_pool.tile([128, RC * 128], BF16, tag="y")

        G = G_cur

        # ---- recurrent matmul: (i,f,o) rounds, then sigmoid, then g~ rounds
        for r in range(4):
            for j in range(4):
                nc.tensor.matmul(
                    G[32 * j:32 * j + 8, 0:384],
                    h_cur[:, 8 * r:8 * r + 8],
                    whh_t[:, 2048 * r + 512 * j: 2048 * r + 512 * j + 384],
                    start=False, stop=False,
                    tile_position=(0, 32 * j),
                    skip_group_check=True,
                )
        S = ew_pool.tile([128, 384], BF16, tag="sifo")
        nc.scalar.activation(S[0:104, :], G[0:104, 0:384], AF.Sigmoid)
        for r in range(4):
            for j in range(4):
                nc.tensor.matmul(
                    G[32 * j:32 * j + 8, 384:512],
                    h_cur[:, 8 * r:8 * r + 8],
                    whh_t[:, 2048 * r + 512 * j + 384: 2048 * r + 512 * (j + 1)],
                    start=False, stop=False,
                    tile_position=(0, 32 * j),
                    skip_group_check=True,
                )
        S_g = ew_pool.tile([128, 128], BF16, tag="sg")
        nc.scalar.activation(S_g[0:104, :], G[0:104, 384:512], AF.Sigmoid)

        # ---- elementwise: c' = sig(f)*c + sig(i)*(2*sig(2g~)-1) ----
        M_t = ew_pool.tile([128, 128], F32, tag="m")
        nc.vector.tensor_mul(M_t[0:104, :], S[0:104, 128:256], c_cur[0:104, :])
        Q_t = ew_pool.tile([128, 128], BF16, tag="q")
        nc.vector.scalar_tensor_tensor(
            Q_t[0:104, :], S_g[0:104, :], -0.5, S[0:104, 0:128],
            ALU.add, ALU.mult)
        c_new = state_pool.tile([128, 128], F32, tag="c")
        nc.vector.scalar_tensor_tensor(
            c_new[0:104, :], Q_t[0:104, :], 2.0, M_t[0:104, :],
            ALU.mult, ALU.add)

        T_c = ew_pool.tile([128, 128], BF16, tag="tc")
        nc.scalar.activation(T_c[0:104, :], c_new[0:104, :], AF.Tanh)

        nc.vector.tensor_mul(
            y_tile[0:104, tl * 128:(tl + 1) * 128], S[0:104, 256:384],
            T_c[0:104, :]
        )

        # ---- PE filler while the EW chain runs: next idents + 1 ph1 mm ----
        if t + 1 < T_:
            G_cur = do_idents(t + 1)
        drain_ph1(2 if s < 4 else 1)

        # ---- transpose h' for next step's lhsT ----
        h_new = state_pool.tile([128, 32], BF16, tag="hT")
        for r in range(4):
            tp = tp_psum.tile([128, 8], BF16, tag="tp")
            nc.tensor.transpose(
                tp[:, 0:8],
                y_tile[32 * r:32 * r + 8, tl * 128:(tl + 1) * 128],
                I4_t[32 * r:32 * r + 8, :],
                tile_position=(32 * r, 0),
            )
            nc.vector.tensor_copy(h_new[:, 8 * r:8 * r + 8], tp[:, 0:8])

        h_cur = h_new
        c_cur = c_new

        if tl == RC - 1:
            for q in range(4):
                nc.sync.dma_start(y_d[rc, q], y_tile[32 * q:32 * q + 8, :])


# ---------------- host side ----------------

def _prep_inputs_core(x_core, wih_r, whh_r, bias_r, E_np, I4_np, ones_np, T_):
    nch = T_ // CH
    xx = x_core.reshape(nch, CH, BL, 4, 128)          # [ch, t_lo, b, k, p]
    xT = np.ascontiguousarray(xx.transpose(0, 4, 3, 1, 2).reshape(nch, 128, 512))
    return {
        "xT": xT.astype(bf),
        "wih": wih_r, "whh": whh_r, "bias": bias_r,
        "E": E_np, "I4": I4_np, "ones": ones_np,
    }


def prep_all_inputs(x, W_ih, W_hh, b_ih, b_hh, T_=T):
    # weight reorder: free col = 2048*k + g',  g' = 512*q + 128*ty' + h_lo
    #   value = W[512*ty + 128*q + h_lo, 128*k + p]
    # device type order ty' = (i, f, o, g~); the g~ block is pre-doubled so
    # the device computes sigmoid(2*g~) and recovers tanh(g~) = 2*sig(2g~)-1
    PERM = [0, 1, 3, 2]                               # (i, f, o, g)
    GSCALE = np.ones((4, 1, 1, 1, 1), np.float32)
    GSCALE[3] = 2.0                                   # g~ (last in device order)

    def reorder_w(W):
        wr = W.reshape(4, 4, 128, 4, 128)[PERM] * GSCALE  # [ty', q, h_lo, k, p]
        return np.ascontiguousarray(
            wr.transpose(4, 3, 1, 0, 2).reshape(128, 4 * 2048)).astype(bf)

    wih_r = reorder_w(W_ih)
    whh_r = reorder_w(W_hh)
    bias_r = np.ascontiguousarray(
        ((b_ih + b_hh).reshape(4, 4, 128)[PERM] * GSCALE[:, 0, 0])
        .transpose(1, 0, 2).reshape(1, 2048)
    ).astype(bf)

    E_np = np.zeros((128, 512), np.float32)
    for s in range(16):
        for b in range(8):
            E_np[8 * s + b, 32 * s + b] = 1.0
    E_np = E_np.astype(bf)
    I4_np = np.zeros((128, 8), np.float32)
    for r in range(4):
        for b in range(8):
            I4_np[32 * r + b, b] = 1.0
    I4_np = I4_np.astype(bf)
    ones_np = np.ones((1, 128), np.float32).astype(bf)

    in_maps = []
    for c in range(NCORES):
        x_core = np.asarray(x[:, BL * c:BL * (c + 1), :], np.float32)
        in_maps.append(_prep_inputs_core(
            x_core, wih_r, whh_r, bias_r, E_np, I4_np, ones_np, T_))
    return in_maps


def decode_output(results, T_=T):
    nrc = T_ // RC
    y = np.zeros((T_, B, H), np.float32)
    for c in range(NCORES):
        r = np.asarray(results[c]["y"]).astype(np.float32)
        r = r.reshape(nrc, 4, 8, RC, 128)             # [rc, q, b, t_lo, h_lo]
        y[:, BL * c:BL * (c + 1), :] = (
            r.transpose(0, 3, 2, 1, 4).reshape(T_, 8, 512))
    return y


_prog_cache = {}


def kernel(x, W_ih, W_hh, b_ih, b_hh):
    if T not in _prog_cache:
        _prog_cache[T] = build_program(T)
    nc = _prog_cache[T]
    in_maps = prep_all_inputs(np.asarray(x, np.float32),
                              np.asarray(W_ih, np.float32),
                              np.asarray(W_hh, np.float32),
                              np.asarray(b_ih, np.float32),
                              np.asarray(b_hh, np.float32))
    from concourse.bass_utils import run_bass_kernel_spmd
    res = run_bass_kernel_spmd(nc, in_maps, core_ids=list(range(NCORES)))
    return decode_output(res.results)


# revision 21
# speedup vs baseline: 1.1630x; 1.1630x over previous
"""LSTM (T=512, B=64, I=H=512) on 8 TRN2 NeuronCores, batch-data-parallel.

Per core (batch slice of 8):
  phase-1 (interleaved): x_proj = x @ W_ih.T + b  -> bf16 ring, 16 steps/chunk
  recurrence, per step:
    PSUM gates G [128, 512] fp32, "quartered" layout: quarter j (h-dims
    128j..128j+128, gate types i,f,o,g along free in 128-blocks) lives at
    partitions 32j+b.
      - 4 slot-select identity matmuls inject x_proj[t] (start=True, M=32
        zero-padded -> initializes all 128 partitions)
      - 4 k-rounds x 4 col-tiled concurrent matmuls accumulate h@W_hh.T
    ACT: sigmoid(i,f,2g~) / sigmoid(o) from PSUM -> bf16 SBUF
    DVE: Q = (sig(2g~)-0.5)*sig(i); GP: M = sig(f)*c;
    DVE: c' = 2Q + M (fp32); ACT: tanh(c')
    per-quarter: DVE ymul h'_q = sig(o)_q*tanh_q -> y ring; PE transpose;
    DVE copy -> hT quarter; next step's k-round r starts right after
    copy-q_r (quarter-pipelined tail).
  output ring DMA'd to HBM bf16 every 64 steps; host converts to fp32.
"""
import numpy as np
import ml_dtypes
from contextlib import ExitStack

import concourse.bass as bass
import concourse.tile as tile
from concourse import bacc, mybir
from concourse._compat import with_exitstack

BF16 = mybir.dt.bfloat16
F32 = mybir.dt.float32
AF = mybir.ActivationFunctionType
ALU = mybir.AluOpType
bf = ml_dtypes.bfloat16

T, B, I, H = 512, 64, 512, 512
NCORES = 8
BL = B // NCORES          # 8 batch rows per core
CH = 16                   # phase-1 chunk (timesteps)
RC = 64                   # output ring chunk (timesteps)
LOOKAHEAD = 2             # xp chunks produced ahead


def build_program(T_=T):
    nch = T_ // CH
    nrc = T_ // RC
    nc = bacc.Bacc("TRN2", target_bir_lowering=False, debug=False)

    xT_d = nc.dram_tensor("xT", [nch, 128, 512], BF16, kind="ExternalInput").ap()
    wih_d = nc.dram_tensor("wih", [128, 4 * 2048], BF16, kind="ExternalInput").ap()
    whh_d = nc.dram_tensor("whh", [128, 4 * 2048], BF16, kind="ExternalInput").ap()
    bias_d = nc.dram_tensor("bias", [1, 2048], BF16, kind="ExternalInput").ap()
    E_d = nc.dram_tensor("E", [128, 512], BF16, kind="ExternalInput").ap()
    I4_d = nc.dram_tensor("I4", [128, 8], BF16, kind="ExternalInput").ap()
    ones_d = nc.dram_tensor("ones", [1, 128], BF16, kind="ExternalInput").ap()
    y_d = nc.dram_tensor("y", [nrc, 4, 8, RC * 128], BF16, kind="ExternalOutput").ap()

    with tile.TileContext(nc) as tc:
        _kernel(tc, y_d, xT_d, wih_d, whh_d, bias_d, E_d, I4_d, ones_d, T_)
    nc.compile()
    return nc


@with_exitstack
def _kernel(ctx: ExitStack, tc: tile.TileContext, y_d, xT_d, wih_d, whh_d,
            bias_d, E_d, I4_d, ones_d, T_):
    nc = tc.nc
    nch = T_ // CH

    const_pool = ctx.enter_context(tc.tile_pool(name="const", bufs=1))
    w_pool = ctx.enter_context(tc.tile_pool(name="weights", bufs=1))
    xT_pool = ctx.enter_context(tc.tile_pool(name="xT", bufs=3))
    xp_pool = ctx.enter_context(tc.tile_pool(name="xp", bufs=LOOKAHEAD + 1))
    y_pool = ctx.enter_context(tc.tile_pool(name="yring", bufs=2))
    ew_pool = ctx.enter_context(tc.tile_pool(name="ew", bufs=2))
    state_pool = ctx.enter_context(tc.tile_pool(name="state", bufs=2))

    # bufs=3 on G: with 2, the o-gate matmuls of step t WAR-wait on
    # sig_o(t-2)'s read of the recycled bank, head-of-line-blocking the
    # in-order Tensor queue ~2us every step.
    g_psum = ctx.enter_context(tc.tile_pool(name="gpsum", bufs=3, space="PSUM"))
    xp_psum = ctx.enter_context(tc.tile_pool(name="xppsum", bufs=1, space="PSUM"))
    tp_psum = ctx.enter_context(tc.tile_pool(name="tppsum", bufs=4, space="PSUM"))

    # ---- load constants/weights ----
    wih_t = w_pool.tile([128, 4 * 2048], BF16)
    nc.sync.dma_start(wih_t[:], wih_d[:])
    whh_t = w_pool.tile([128, 4 * 2048], BF16)
    nc.sync.dma_start(whh_t[:], whh_d[:])
    bias_t = const_pool.tile([1, 2048], BF16)
    nc.sync.dma_start(bias_t[:], bias_d[:])
    E_t = const_pool.tile([128, 512], BF16)
    nc.sync.dma_start(E_t[:], E_d[:])
    I4_t = const_pool.tile([128, 8], BF16)
    nc.sync.dma_start(I4_t[:], I4_d[:])
    ones_t = const_pool.tile([1, 128], BF16)
    nc.sync.dma_start(ones_t[:], ones_d[:])

    # ---- initial state ----
    h_cur = state_pool.tile([128, 32], BF16, tag="hT")
    nc.gpsimd.memset(h_cur[:], 0.0)
    c_cur = state_pool.tile([128, 128], F32, tag="c")
    nc.gpsimd.memset(c_cur[:], 0.0)

    xp_tiles = {}
    xt_tiles = {}
    ph1_mms = []              # pending phase-1 matmul thunks, 1 drained/step

    def xp_dma(ch):
        xt = xT_pool.tile([128, 512], BF16, tag="xchunk")
        nc.sync.dma_start(xt[:], xT_d[ch])
        xt_tiles[ch] = xt

    def queue_xp_chunk(ch):
        """queue phase-1 matmuls for chunk ch: 4 quarters x 5 mms + copy."""
        xp_new = xp_pool.tile([128, 2048], BF16, tag="xp")
        xp_tiles[ch] = xp_new
        for qf in range(4):
            ps_box = []
            g0 = qf * 512

            def mk_mm(k, qf=qf, g0=g0, ch=ch, ps_box=ps_box):
                def run():
                    if k == 0:
                        ps_new = xp_psum.tile([128, 512], F32, tag="xpp")
                        ps_box.append(ps_new)
                    ps = ps_box[0]
                    if k < 4:
                        nc.tensor.matmul(
                            ps[:, :],
                            xt_tiles[ch][:, k * 128:(k + 1) * 128],
                            wih_t[:, 2048 * k + g0: 2048 * k + g0 + 512],
                            start=(k == 0), stop=False,
                        )
                    else:
                        nc.tensor.matmul(
                            ps[:, :], ones_t[:, :], bias_t[:, g0:g0 + 512],
                            start=False, stop=True,
                        )
                        nc.vector.tensor_copy(
                            xp_tiles[ch][:, g0:g0 + 512], ps[:])
                return run

            for k in range(5):
                ph1_mms.append(mk_mm(k))

    def drain_ph1(n):
        for _ in range(min(n, len(ph1_mms))):
            ph1_mms.pop(0)()

    def do_idents(t_next):
        """allocate G psum tile for step t_next + slot-select x_proj inject.
        One [128, 512] tile: cols 0:384 = i,f,g~; cols 384:512 = o. The
        region-based dep tracking lets sigmoid(ifg) ignore o-col writes."""
        s = t_next % CH
        xp = xp_tiles[t_next // CH]
        G = g_psum.tile([128, 512], F32, tag="G")
        for j in range(4):
            nc.tensor.matmul(
                G[32 * j:32 * j + 32, 0:384],
                E_t[:, 32 * s:32 * s + 32],
                xp[:, 512 * j:512 * j + 384],
                start=True, stop=True,
                tile_position=(0, 32 * j),
                skip_group_check=(j > 0),
            )
        # start=False: the ifg idents' start=True already pending-zeroes the
        # full 2KB bank region per partition (ZERO_REGION_SIZE), o included.
        for j in range(4):
            nc.tensor.matmul(
                G[32 * j:32 * j + 32, 384:512],
                E_t[:, 32 * s:32 * s + 32],
                xp[:, 512 * j + 384:512 * (j + 1)],
                start=False, stop=True,
                tile_position=(0, 32 * j),
                skip_group_check=True,
            )
        return G

    # prologue: first two x_proj chunks + idents for step 0
    for ch in range(min(LOOKAHEAD, nch)):
        xp_dma(ch)
        queue_xp_chunk(ch)
        drain_ph1(20)
    G_cur = do_idents(0)

    y_tile = None
    for t in range(T_):
        s = t % CH
        ch = t // CH
        tl = t % RC
        rc = t // RC

        if s == 0 and ch + LOOKAHEAD < nch:
            xp_dma(ch + LOOKAHEAD)
            queue_xp_chunk(ch + LOOKAHEAD)
        if tl == 0:
            y_tile = y_pool.tile([128, RC * 128], BF16, tag="y")

        G = G_cur

        # ---- recurrent matmul: (i,f,2g~) rounds, then sigmoid, then o rounds
        for r in range(4):
            for j in range(4):
                nc.tensor.matmul(
                    G[32 * j:32 * j + 8, 0:384],
                    h_cur[:, 8 * r:8 * r + 8],
                    whh_t[:, 2048 * r + 512 * j: 2048 * r + 512 * j + 384],
                    start=False, stop=False,
                    tile_position=(0, 32 * j),
                    skip_group_check=True,
                )
        S = ew_pool.tile([128, 384], BF16, tag="sifg")
        nc.scalar.activation(S[0:104, :], G[0:104, 0:384], AF.Sigmoid)
        for r in range(4):
            for j in range(4):
                nc.tensor.matmul(
                    G[32 * j:32 * j + 8, 384:512],
                    h_cur[:, 8 * r:8 * r + 8],
                    whh_t[:, 2048 * r + 512 * j + 384: 2048 * r + 512 * (j + 1)],
                    start=False, stop=False,
                    tile_position=(0, 32 * j),
                    skip_group_check=True,
                )
        S_o = ew_pool.tile([128, 128], BF16, tag="so")
        nc.scalar.activation(S_o[0:104, :], G[0:104, 384:512], AF.Sigmoid)

        # ---- elementwise: c' = sig(f)*c + sig(i)*(2*sig(2g~)-1) ----
        Q_t = ew_pool.tile([128, 128], BF16, tag="q")
        nc.vector.scalar_tensor_tensor(
            Q_t[0:104, :], S[0:104, 256:384], -0.5, S[0:104, 0:128],
            ALU.add, ALU.mult)
        M_t = ew_pool.tile([128, 128], F32, tag="m")
        nc.gpsimd.tensor_mul(M_t[0:104, :], S[0:104, 128:256], c_cur[0:104, :])
        c_new = state_pool.tile([128, 128], F32, tag="c")
        nc.vector.scalar_tensor_tensor(
            c_new[0:104, :], Q_t[0:104, :], 2.0, M_t[0:104, :],
            ALU.mult, ALU.add)

        T_c = ew_pool.tile([128, 128], BF16, tag="tc")
        nc.scalar.activation(T_c[0:104, :], c_new[0:104, :], AF.Tanh)

        nc.vector.tensor_mul(
            y_tile[0:104, tl * 128:(tl + 1) * 128], S_o[0:104, :], T_c[0:104, :]
        )

        # ---- PE filler while the EW chain runs: next idents + 1 ph1 mm ----
        if t + 1 < T_:
            G_cur = do_idents(t + 1)
        drain_ph1(2 if s < 4 else 1)

        # ---- transpose h' for next step's lhsT ----
        h_new = state_pool.tile([128, 32], BF16, tag="hT")
        for r in range(4):
            tp = tp_psum.tile([128, 8], BF16, tag="tp")
            nc.tensor.transpose(
                tp[:, 0:8],
                y_tile[32 * r:32 * r + 8, tl * 128:(tl + 1) * 128],
                I4_t[32 * r:32 * r + 8, :],
                tile_position=(32 * r, 0),
            )
            nc.vector.tensor_copy(h_new[:, 8 * r:8 * r + 8], tp[:, 0:8])

        h_cur = h_new
        c_cur = c_new

        if tl == RC - 1:
            for q in range(4):
                nc.sync.dma_start(y_d[rc, q], y_tile[32 * q:32 * q + 8, :])


# ---------------- host side ----------------

def _prep_inputs_core(x_core, wih_r, whh_r, bias_r, E_np, I4_np, ones_np, T_):
    nch = T_ // CH
    xx = x_core.reshape(nch, CH, BL, 4, 128)          # [ch, t_lo, b, k, p]
    xT = np.ascontiguousarray(xx.transpose(0, 4, 3, 1, 2).reshape(nch, 128, 512))
    return {
        "xT": xT.astype(bf),
        "wih": wih_r, "whh": whh_r, "bias": bias_r,
        "E": E_np, "I4": I4_np, "ones": ones_np,
    }


def prep_all_inputs(x, W_ih, W_hh, b_ih, b_hh, T_=T):
    # weight reorder: free col = 2048*k + g',  g' = 512*q + 128*ty + h_lo
    #   value = W[512*ty + 128*q + h_lo, 128*k + p]
    # natural type order (i, f, g, o); the g~ block is pre-doubled so the
    # device computes sigmoid(2*g~) and recovers tanh(g~) = 2*sig(2g~)-1
    GSCALE = np.ones((4, 1, 1, 1, 1), np.float32)
    GSCALE[2] = 2.0

    def reorder_w(W):
        wr = W.reshape(4, 4, 128, 4, 128) * GSCALE    # [ty, q, h_lo, k, p]
        return np.ascontiguousarray(
            wr.transpose(4, 3, 1, 0, 2).reshape(128, 4 * 2048)).astype(bf)

    wih_r = reorder_w(W_ih)
    whh_r = reorder_w(W_hh)
    bias_r = np.ascontiguousarray(
        ((b_ih + b_hh).reshape(4, 4, 128) * GSCALE[:, 0, 0])
        .transpose(1, 0, 2).reshape(1, 2048)
    ).astype(bf)

    E_np = np.zeros((128, 512), np.float32)
    for s in range(16):
        for b in range(8):
            E_np[8 * s + b, 32 * s + b] = 1.0
    E_np = E_np.astype(bf)
    I4_np = np.zeros((128, 8), np.float32)
    for r in range(4):
        for b in range(8):
            I4_np[32 * r + b, b] = 1.0
    I4_np = I4_np.astype(bf)
    ones_np = np.ones((1, 128), np.float32).astype(bf)

    in_maps = []
    for c in range(NCORES):
        x_core = np.asarray(x[:, BL * c:BL * (c + 1), :], np.float32)
        in_maps.append(_prep_inputs_core(
            x_core, wih_r, whh_r, bias_r, E_np, I4_np, ones_np, T_))
    return in_maps


def decode_output(results, T_=T):
    nrc = T_ // RC
    y = np.zeros((T_, B, H), np.float32)
    for c in range(NCORES):
        r = np.asarray(results[c]["y"]).astype(np.float32)
        r = r.reshape(nrc, 4, 8, RC, 128)             # [rc, q, b, t_lo, h_lo]
        y[:, BL * c:BL * (c + 1), :] = (
            r.transpose(0, 3, 2, 1, 4).reshape(T_, 8, 512))
    return y


_prog_cache = {}


def kernel(x, W_ih, W_hh, b_ih, b_hh):
    if T not in _prog_cache:
        _prog_cache[T] = build_program(T)
    nc = _prog_cache[T]
    in_maps = prep_all_inputs(np.asarray(x, np.float32),
                              np.asarray(W_ih, np.float32),
                              np.asarray(W_hh, np.float32),
                              np.asarray(b_ih, np.float32),
                              np.asarray(b_hh, np.float32))
    from concourse.bass_utils import run_bass_kernel_spmd
    res = run_bass_kernel_spmd(nc, in_maps, core_ids=list(range(NCORES)))
    return decode_output(res.results)
